# revision 1
# baseline (speedup 1.0000x reference)
"""Trainium2 Bass kernel for nn_CustomTSPInitEmbedding.

Reference computation (per batch b of B=16, N=2000 2-D points):
  diff[i,j]  = locs[j] - locs[i]
  dists      = ||diff||, diag=inf
  idx        = 10 nearest neighbors per node (by distance, first-index ties)
  rel        = diff gathered at idx                       (N, 10, 2)
  feats      = [locs, rel.reshape(N,20)]                  (N, 22)
  out        = feats @ W.T + b                            (N, 128)

Sharding: batch across 8 cores (2 batches per core), fully data parallel.

Per-core kernel, for each batch and each 128-row tile:
  1. PE matmul with augmented features gives -d^2 for the whole row-tile:
       -d2[i,j] = [-|xi|^2, 2xi_x, 2xi_y, -1] . [1, xj_x, xj_y, |xj|^2]
     (tables precomputed on host; ~2e-7 abs noise from f32 cancellation)
  2. diag masked via affine_select; DVE max8/max_index/match_replace ISA ops
     select top-16 candidate indices (noisy top-16 always covers exact top-10)
  3. gpsimd ap_gather fetches the candidate coords from an SBUF-replicated
     locs table (indices pre-wrapped per 16-partition core block via a DRAM
     round trip); exact rel/d^2 recomputed with the reference's own f32 op
     order; tiny max8 pass re-ranks exactly
  4. rank-k rel vectors extracted by value-matching (scalar_tensor_tensor
     with row-sum accumulator) straight into the feature tile; PE transpose +
     matmul against host-prepped [W.T; b] with a constant-1 feature gives the
     output tile
"""

import numpy as np

import concourse.bass as bass
import concourse.bacc as bacc
import concourse.mybir as mybir
from concourse.tile import TileContext
from concourse import bass_utils

F32 = mybir.dt.float32
U16 = mybir.dt.uint16
I16 = mybir.dt.int16

B, N, D_EMB, K, NCAND = 16, 2000, 128, 10, 16
NPAD = 2048                      # N padded to a multiple of 128
BPC = 2                          # batches per core
NCORES = 8
NTILES = NPAD // 128             # row tiles per batch
NEG_BIG = -3.0e38


def build_nc():
    nc = bacc.Bacc(None, target_bir_lowering=False)

    locs = nc.dram_tensor("locs", [BPC * NPAD, 2], F32, kind="ExternalInput")
    atab = nc.dram_tensor("atab", [BPC, 4, NPAD], F32, kind="ExternalInput")
    btab = nc.dram_tensor("btab", [BPC, 4, NPAD], F32, kind="ExternalInput")
    # interleaved x0,y0,x1,y1,... per batch, for the replicated SBUF table
    ltab = nc.dram_tensor("ltab", [BPC, 2 * N], F32, kind="ExternalInput")
    ones = nc.dram_tensor("ones", [1, 128], F32, kind="ExternalInput")
    wtb = nc.dram_tensor("wtb", [23, D_EMB], F32, kind="ExternalInput")
    idm = nc.dram_tensor("idm", [128, 128], F32, kind="ExternalInput")
    out = nc.dram_tensor("out", [BPC, N, D_EMB], F32, kind="ExternalOutput")

    with TileContext(nc) as tc:
        with (
            tc.tile_pool(name="const", bufs=1) as cpool,
            tc.tile_pool(name="d2", bufs=2) as d2pool,
            tc.tile_pool(name="small", bufs=4) as spool,
            tc.tile_pool(name="gath", bufs=2) as gpool,
            tc.tile_pool(name="feats", bufs=3) as fpool,
            tc.tile_pool(name="psum_d2", bufs=1, space="PSUM") as pd2,
            tc.tile_pool(name="psum_t", bufs=1, space="PSUM") as ptp,
            tc.tile_pool(name="psum_o", bufs=2, space="PSUM") as pop,
            tc.tile_pool(name="psum_l", bufs=1, space="PSUM") as plp,
            tc.tile_pool(name="dram", bufs=4, space="DRAM") as dpool,
        ):
            # --- constants, loaded once
            wtb_sb = cpool.tile([23, D_EMB], F32, tag="wtb")
            nc.sync.dma_start(wtb_sb[:], wtb[:])
            idm_sb = cpool.tile([128, 128], F32, tag="idm")
            nc.sync.dma_start(idm_sb[:], idm[:])
            ones_sb = cpool.tile([1, 128], F32, tag="ones")
            nc.sync.dma_start(ones_sb[:], ones[:])
            atab_sb = cpool.tile([4, BPC * NPAD], F32, tag="atab")
            nc.sync.dma_start(
                atab_sb[:].rearrange("f (b n) -> f b n", b=BPC),
                atab[:].rearrange("b f n -> f b n"),
            )
            btab_sb = cpool.tile([4, BPC * NPAD], F32, tag="btab")
            nc.sync.dma_start(
                btab_sb[:].rearrange("f (b n) -> f b n", b=BPC),
                btab[:].rearrange("b f n -> f b n"),
            )
            ltab_sb = cpool.tile([1, BPC * 2 * N], F32, tag="ltab")
            nc.sync.dma_start(
                ltab_sb[:].rearrange("o (b n) -> o b n", b=BPC), ltab[:])

            # --- replicated locs tables, one per batch: [128, N, 2]
            tabs = []
            for bi in range(BPC):
                tab = cpool.tile([128, N * 2], F32, tag=f"loctab{bi}")
                for c0 in range(0, 2 * N, 512):
                    cw = min(512, 2 * N - c0)
                    tp = plp.tile([128, 512], F32, tag="tbuild")
                    nc.tensor.matmul(
                        tp[:, 0:cw], ones_sb[:],
                        ltab_sb[:, bi * 2 * N + c0: bi * 2 * N + c0 + cw],
                        start=True, stop=True)
                    nc.scalar.copy(tab[:, c0:c0 + cw], tp[:, 0:cw])
                tabs.append(tab)

            for bi in range(BPC):
                asb = atab_sb[:, bi * NPAD:(bi + 1) * NPAD]
                bsb = btab_sb[:, bi * NPAD:(bi + 1) * NPAD]
                tab = tabs[bi]
                for tt in range(NTILES):
                    r0 = 128 * tt
                    rows = min(128, N - r0)      # valid rows (80 on last tile)

                    # --- 1. -d^2 row-tile via PE
                    d2ps = pd2.tile([128, 2048], F32, tag="d2ps")
                    for c0 in range(0, N, 512):
                        cw = min(512, N - c0)
                        nc.tensor.matmul(
                            d2ps[:, c0:c0 + cw],
                            asb[:, r0:r0 + 128],
                            bsb[:, c0:c0 + cw],
                            start=True, stop=True,
                        )
                    d2 = d2pool.tile([128, N], F32, tag="d2")
                    nc.scalar.copy(d2[:], d2ps[:, 0:N])

                    # --- 2. mask diagonal, select top-16 noisy candidates
                    dw = min(128, N - r0)
                    nc.gpsimd.affine_select(
                        d2[:, r0:r0 + dw], d2[:, r0:r0 + dw],
                        pattern=[[1, dw]], base=0, channel_multiplier=-1,
                        compare_op=mybir.AluOpType.not_equal, fill=NEG_BIG,
                    )
                    v = spool.tile([128, 16], F32, tag="v")
                    ci = spool.tile([128, NCAND], U16, tag="ci")
                    nc.vector.max(v[:, 0:8], d2[:])
                    nc.vector.max_index(ci[:, 0:8], v[:, 0:8], d2[:])
                    nc.vector.match_replace(d2[:], v[:, 0:8], d2[:], NEG_BIG)
                    nc.vector.max(v[:, 8:16], d2[:])
                    nc.vector.max_index(ci[:, 8:16], v[:, 8:16], d2[:])

                    # --- 3. gather candidate coords via ap_gather. Each
                    # gpsimd core c reads its index list from partitions
                    # [16c, 16c+16): list entry q comes from partition
                    # 16c + q%16, slot q//16 — so ci itself IS the index
                    # buffer for the list order q = cand*16 + r, and
                    # og[p, 16*cand + p%16, :] = tab[ci[p, cand]].
                    og = gpool.tile([128, 256, 2], F32, tag="og")
                    # pre-init so the sim's shadow-memory checker accepts the
                    # partition-strided extraction reads below
                    nc.gpsimd.memset(og[:], 0.0)
                    nc.gpsimd.ap_gather(
                        out_ap=og[:], in_ap=tab[:].rearrange(
                            "p (n d) -> p n d", d=2),
                        idxs_ap=ci[:].bitcast(I16),
                        channels=128, num_elems=N, d=2, num_idxs=256)
                    # extraction is per-partition-residue strided -> DMA only
                    cc = spool.tile([128, NCAND, 2], F32, tag="cc")
                    for r in range(16):
                        src = og[r:128:16, r:256:16, :]
                        eng = nc.sync if r % 2 == 0 else nc.scalar
                        eng.dma_start(cc[r:128:16, :, :], src)

                    # --- 4. exact rel + d^2 (reference f32 op order), re-rank
                    feats = fpool.tile([128, 23], F32, tag="feats")
                    nc.sync.dma_start(
                        feats[:, 0:2],
                        locs[bi * NPAD + r0: bi * NPAD + r0 + 128, :],
                    )
                    nc.vector.tensor_scalar(
                        cc[:, :, 0:1], cc[:, :, 0:1], feats[:, 0:1], None,
                        op0=mybir.AluOpType.subtract)
                    nc.vector.tensor_scalar(
                        cc[:, :, 1:2], cc[:, :, 1:2], feats[:, 1:2], None,
                        op0=mybir.AluOpType.subtract)
                    sq = spool.tile([128, NCAND, 2], F32, tag="sq")
                    nc.vector.tensor_tensor(
                        out=sq[:], in0=cc[:], in1=cc[:],
                        op=mybir.AluOpType.mult)
                    d2c = spool.tile([128, NCAND], F32, tag="d2c")
                    nc.vector.tensor_reduce(
                        out=d2c[:], in_=sq[:], axis=mybir.AxisListType.X,
                        op=mybir.AluOpType.add)
                    nc.vector.tensor_scalar(
                        d2c[:], d2c[:], -1.0, None, op0=mybir.AluOpType.mult)

                    v2 = spool.tile([128, 16], F32, tag="v2")
                    d2m = spool.tile([128, NCAND], F32, tag="d2m")
                    nc.vector.max(v2[:, 0:8], d2c[:])
                    nc.vector.match_replace(d2m[:], v2[:, 0:8], d2c[:],
                                            NEG_BIG)
                    nc.vector.max(v2[:, 8:16], d2m[:])

                    # --- 5. rank-k rel via value match + row-sum accumulate
                    for k in range(K):
                        for xy in range(2):
                            nc.vector.scalar_tensor_tensor(
                                out=sq[:, :, xy:xy + 1],
                                in0=d2c[:].unsqueeze(2),
                                in1=cc[:, :, xy:xy + 1],
                                scalar=v2[:, k:k + 1],
                                op0=mybir.AluOpType.is_equal,
                                op1=mybir.AluOpType.mult,
                                accum_out=feats[:, 2 + 2 * k + xy:
                                                3 + 2 * k + xy])
                    nc.vector.memset(feats[:, 22:23], 1.0)

                    # --- 6. linear layer
                    ftp = ptp.tile([23, 128], F32, tag="ftp")
                    nc.tensor.transpose(ftp[:], feats[:], idm_sb[:])
                    fts = fpool.tile([23, 128], F32, tag="fts")
                    nc.scalar.copy(fts[:], ftp[:])
                    op = pop.tile([128, D_EMB], F32, tag="op")
                    nc.tensor.matmul(op[:], fts[:], wtb_sb[:],
                                     start=True, stop=True)
                    ob = fpool.tile([128, D_EMB], F32, tag="ob")
                    nc.scalar.copy(ob[:], op[:])
                    nc.sync.dma_start(out[bi, r0:r0 + rows, :], ob[0:rows, :])

    nc.compile()
    return nc


_CACHE: dict = {}


def _prep_core_inputs(locs_np, W, b, core):
    """Host-side input prep for one core (its 2 batches)."""
    f32 = np.float32
    lp = np.empty((BPC, NPAD, 2), dtype=f32)
    at = np.empty((BPC, 4, NPAD), dtype=f32)
    bt = np.empty((BPC, 4, NPAD), dtype=f32)
    for j in range(BPC):
        lb = locs_np[core * BPC + j].astype(f32)
        lp[j, :N] = lb
        lp[j, N:] = lb[0]
        x, y = lp[j, :, 0], lp[j, :, 1]
        nrm = (x * x + y * y).astype(f32)
        at[j, 0] = -nrm
        at[j, 1] = 2.0 * x
        at[j, 2] = 2.0 * y
        at[j, 3] = -1.0
        bt[j, 0] = 1.0
        bt[j, 1] = x
        bt[j, 2] = y
        bt[j, 3] = nrm
    wtb = np.concatenate([W.T.astype(f32), b[None, :].astype(f32)], axis=0)
    return {
        "locs": np.ascontiguousarray(lp.reshape(BPC * NPAD, 2)),
        "atab": at,
        "btab": bt,
        "ltab": np.ascontiguousarray(lp[:, :N, :].reshape(BPC, 2 * N)),
        "ones": np.ones((1, 128), dtype=f32),
        "wtb": np.ascontiguousarray(wtb),
        "idm": np.eye(128, dtype=f32),
    }


def kernel(locs, W, b):
    locs = np.asarray(locs)
    W = np.asarray(W)
    b = np.asarray(b)
    if "nc" not in _CACHE:
        _CACHE["nc"] = build_nc()
    nc = _CACHE["nc"]
    in_maps = [_prep_core_inputs(locs, W, b, c) for c in range(NCORES)]
    res = bass_utils.run_bass_kernel_spmd(nc, in_maps,
                                          core_ids=list(range(NCORES)))
    outs = [res.results[c]["out"] for c in range(NCORES)]
    return np.concatenate(outs, axis=0).astype(np.float32)

